# revision 1
# baseline (speedup 1.0000x reference)
"""Trainium2 Bass kernel for nn_MaxExtractor (masked pairwise-IoU max + union max).

Contract: kernel(**inputs) takes FULL unsharded inputs, returns the FULL [2]
output. Internally shards the batch dim (8 images) across 8 NeuronCores, one
image per core; each core computes [max_prob_t, iou_max_of_its_image]; the
host gathers and averages the per-image iou scalars.

Per-core algorithm (N=4096 preds, M=2048 gts):
  Only preds with class==0 (~50/4096) matter, so the core first compacts
  person preds on-device: mask -> free-dim cumsum (tensor_tensor_scan) ->
  cross-partition exclusive prefix (triangular matmul) -> ranks -> one-hot
  -> accumulating PE matmuls gather person boxes into [128, 4].
  Slot layout (K=64): partition p holds person (p % 64) and gt-half (p // 64),
  so pairwise tiles are [128, 512] over 2 gt blocks.
  Gt/pred boxes are pre-split on the host into bf16 hi+lo parts (lossless to
  ~2^-18); two accumulating bf16 matmuls reconstruct fp32 in PSUM at the PE's
  1 cycle/row bf16 rate, dodging the 4 cycles/row fp32 tax.
  Pairwise: iw = min(px2,gx2)-max(px1,gx1) etc. via fused DVE ops; instead of
  iou = inter/uni per pair, rank by r = inter/(area_p+area_g+eps) which is a
  monotone transform of iou (iou = r/(1-r)); one reciprocal_approx_fast per
  block replaces per-pair division. Final: partition all-reduce max,
  iou = r*/(1-r*).
"""

import sys

sys.path.insert(0, "/opt/trn_rl_repo")

import contextlib

import numpy as np

import concourse.bacc as bacc
import concourse.mybir as mybir
from concourse import bass_isa
from concourse.tile import TileContext

F32 = mybir.dt.float32
BF16 = mybir.dt.bfloat16
I32 = mybir.dt.int32
Alu = mybir.AluOpType
Act = mybir.ActivationFunctionType

N = 4096  # preds per image
M = 2048  # gts per image
B = 8  # images == cores
U = 4096  # union entries
BIG = 1.0e30
EPS = 1.0e-9
FDB = 512  # gt-block free size for the pairwise phase (1 PSUM bank)
NCH = 32  # pred chunks of 128 (compaction contract dim)


def split_hi_lo(x: np.ndarray):
    """bf16 hi+lo decomposition of fp32 data, exact to ~2^-18 relative."""
    bf16 = mybir.dt.np(BF16)
    hi = x.astype(bf16)
    lo = (x - hi.astype(np.float32)).astype(bf16)
    return hi, lo


def build_kernel(K: int):
    """Build the per-core Bass module. K = person-slot count (64 or 128)."""
    assert K in (64, 128)
    nhalf = 128 // K  # gt-halves packed along partitions
    nblk = M // (nhalf * FDB)  # sequential gt blocks
    GW = 4 * M // nhalf  # gt row width (elems per half)

    nc = bacc.Bacc("TRN2", target_bir_lowering=False, debug=False)

    # packed inputs (host-side interleave): misc = [pred_classes | union_scores
    # | union_classes] as u32 rows; pb_hl/gt_hl = bf16 hi/lo interleaved per box
    misc = nc.dram_tensor("misc", [3, N], mybir.dt.uint32, kind="ExternalInput")
    pb_hl = nc.dram_tensor("pb_hl", [N, 8], BF16, kind="ExternalInput")
    gt_boxes = nc.dram_tensor("gt_boxes", [M, 4], F32, kind="ExternalInput")
    gt_hl = nc.dram_tensor("gt_hl", [M, 8], BF16, kind="ExternalInput")
    out = nc.dram_tensor("out", [2], F32, kind="ExternalOutput")

    # ---- constants baked into the NEFF ----
    # col 0-127: strict lower-tri (partition prefix); col 128-255: one-hot
    # rank targets (slot p matches rank (p % K) + 1) as bf16 pair-packed f32
    tri_np = (np.arange(128)[:, None] < np.arange(128)[None, :]).astype(np.float32)
    iota_np = np.tile(np.arange(1, K + 1, dtype=np.float32), 128 // K)
    iota_bf = np.broadcast_to(iota_np.astype(mybir.dt.np(BF16)), (128, 128))
    iota_as_f32 = np.ascontiguousarray(iota_bf).view(np.uint16).astype(np.uint32)
    merged = np.concatenate(
        [tri_np.view(np.uint32), (iota_as_f32[:, 0::2] | (iota_as_f32[:, 1::2] << 16))],
        axis=1,
    )  # [128, 192] u32: tri | bf16-packed iota
    sel_np = np.zeros((128, 128), dtype=np.float32)
    for p in range(128):
        sel_np[(p // K) % nhalf, p] = 1.0
    sel16_bits = np.ascontiguousarray(sel_np.astype(mybir.dt.np(BF16))).view(np.uint16).astype(np.uint32)
    sel16_packed = sel16_bits[:, 0::2] | (sel16_bits[:, 1::2] << 16)
    merged = np.concatenate(
        [merged, sel_np.view(np.uint32), sel16_packed], axis=1
    )  # [128, 384] u32: tri | iota | sel_f32 | sel_bf16
    const_merged = nc.inline_tensor(merged.astype(np.uint32), "consts")

    with TileContext(nc) as tc:
        ctx = contextlib.ExitStack()
        with ctx:
            const_pool = ctx.enter_context(tc.tile_pool(name="const", bufs=1))
            sb = ctx.enter_context(tc.tile_pool(name="sbuf", bufs=1))
            wrk = ctx.enter_context(tc.tile_pool(name="wrk", bufs=2))
            ohp = ctx.enter_context(tc.tile_pool(name="ohp", bufs=32))
            small = ctx.enter_context(tc.tile_pool(name="small", bufs=1))
            ps_g = ctx.enter_context(tc.tile_pool(name="ps_g", bufs=6, space="PSUM"))
            ps_s = ctx.enter_context(tc.tile_pool(name="ps_s", bufs=2, space="PSUM"))

            # ------- loads: few fat DMAs, rank-chain data first ------------
            misc_sb = sb.tile([128, 96], mybir.dt.uint32, tag="misc")
            nc.sync.dma_start(
                out=misc_sb[:],
                in_=misc.ap().rearrange("x (p f) -> p x f", p=128),
            )
            cls_sb = misc_sb[:, 0:32].bitcast(I32)
            uscore = misc_sb[:, 32:64].bitcast(F32)
            ucls = misc_sb[:, 64:96].bitcast(I32)
            cmerged = const_pool.tile([128, 384], mybir.dt.uint32, tag="cmerged")
            nc.scalar.dma_start(out=cmerged[:], in_=const_merged.ap())
            tri_sb = cmerged[:, 0:128].bitcast(F32)
            iota_sb = cmerged[:, 128:192].bitcast(BF16)
            sel_sb = cmerged[0:nhalf, 192:320].bitcast(F32)
            sel16_sb = cmerged[0:nhalf, 320:384].bitcast(BF16)
            ghl = sb.tile([nhalf, 2 * GW], BF16, tag="ghl")
            nc.sync.dma_start(
                out=ghl[:], in_=gt_hl.ap().flatten().rearrange("(q x) -> q x", q=nhalf)
            )
            grow = sb.tile([nhalf, GW], F32, tag="grow")
            nc.scalar.dma_start(
                out=grow[:],
                in_=gt_boxes.ap().flatten().rearrange("(q x) -> q x", q=nhalf),
            )
            pbhl = sb.tile([128, 256], BF16, tag="pbhl")
            nc.sync.dma_start(
                out=pbhl[:], in_=pb_hl.ap().flatten().rearrange("(p f) -> p f", p=128)
            )

            # ------- per-block gt areas on GpSimd (only needs grow) ---------
            ag_row = sb.tile([nhalf, M // nhalf], F32, tag="ag_row")
            for blk in range(nblk):
                g0 = 4 * FDB * blk
                a0 = FDB * blk
                wg_r = wrk.tile([nhalf, FDB], F32, tag="wg_r")
                nc.gpsimd.tensor_sub(
                    wg_r[:], grow[:, g0 + 2 : g0 + 4 * FDB : 4],
                    grow[:, g0 + 0 : g0 + 4 * FDB : 4],
                )
                hg_r = wrk.tile([nhalf, FDB], F32, tag="hg_r")
                nc.gpsimd.tensor_sub(
                    hg_r[:], grow[:, g0 + 3 : g0 + 4 * FDB : 4],
                    grow[:, g0 + 1 : g0 + 4 * FDB : 4],
                )
                nc.gpsimd.tensor_mul(ag_row[:, a0 : a0 + FDB], wg_r[:], hg_r[:])

            # ---------------- person mask + ranks ----------------
            m = small.tile([128, 32], F32, tag="m")
            nc.vector.tensor_scalar(m[:], cls_sb[:], 0, None, Alu.is_equal)
            s = small.tile([128, 32], F32, tag="s")
            nc.vector.tensor_tensor_scan(s[:], m[:], m[:], 0.0, Alu.add, Alu.max)
            pref_ps = ps_s.tile([128, 4], F32, tag="pss")
            nc.tensor.matmul(
                pref_ps[:, 0:1], tri_sb, s[:, 31:32], start=True, stop=True
            )
            q = small.tile([128, 32], F32, tag="q")
            nc.vector.scalar_tensor_tensor(
                q[:], s[:], pref_ps[:, 0:1], m[:], Alu.add, Alu.mult
            )

            # ---------------- compaction: one-hot + matmul gather -----------
            pc_ps = ps_s.tile([128, 4], F32, tag="pss")
            for f in range(NCH):
                oh = ohp.tile([128, 128], BF16, tag="oh")
                nc.vector.tensor_scalar(
                    oh[:], iota_sb, q[:, f : f + 1], None, Alu.is_equal
                )
                nc.tensor.matmul(
                    pc_ps[:], oh[:], pbhl[:, 8 * f : 8 * f + 4],
                    start=(f == 0), stop=False,
                )
                nc.tensor.matmul(
                    pc_ps[:], oh[:], pbhl[:, 8 * f + 4 : 8 * f + 8],
                    start=False, stop=(f == NCH - 1),
                )
            pc = small.tile([128, 4], F32, tag="pcs")
            nc.vector.tensor_copy(pc[:], pc_ps[:])
            px1, py1, px2, py2 = (pc[:, i : i + 1] for i in range(4))
            wp = small.tile([128, 1], F32, tag="wp")
            nc.vector.tensor_sub(wp[:], px2, px1)
            hp = small.tile([128, 1], F32, tag="hp")
            nc.vector.tensor_sub(hp[:], py2, py1)
            ap_eps = small.tile([128, 1], F32, tag="ap_eps")
            nc.vector.scalar_tensor_tensor(
                ap_eps[:], wp[:], EPS, hp[:], Alu.bypass, Alu.mult
            )
            nc.vector.tensor_scalar(ap_eps[:], ap_eps[:], EPS, None, Alu.add)

            # ------- gt coord broadcasts (PE, bf16 hi+lo accumulate) --------
            def bcast_coords(blk):
                g0 = blk * 8 * FDB
                tiles = []
                for c in (0, 2, 1, 3):  # x1, x2, y1, y2
                    gt_ps = ps_g.tile([128, FDB], F32, tag="g")
                    nc.tensor.matmul(
                        gt_ps[:], sel16_sb, ghl[:, g0 + c : g0 + 8 * FDB : 8],
                        start=True, stop=False,
                    )
                    nc.tensor.matmul(
                        gt_ps[:], sel16_sb, ghl[:, g0 + c + 4 : g0 + 8 * FDB : 8],
                        start=False, stop=True,
                    )
                    tiles.append(gt_ps)
                return tiles  # [x1, x2, y1, y2]

            def bcast_area(blk):
                ag_ps = ps_g.tile([128, FDB], F32, tag="g")
                nc.tensor.matmul(
                    ag_ps[:], sel_sb, ag_row[:, blk * FDB : (blk + 1) * FDB],
                    start=True, stop=True,
                )
                return ag_ps

            blk_tiles = {0: (bcast_coords(0), bcast_area(0))}

            # ---------------- union max ----------------
            mu = small.tile([128, U // 128], I32, tag="mu")
            nc.vector.tensor_scalar(mu[:], ucls[:], 0, None, Alu.is_equal)
            um = small.tile([128, U // 128], F32, tag="um")
            nc.vector.memset(um[:], -BIG)
            nc.vector.copy_predicated(um[:], mu[:], uscore[:])
            # ---------------- per gt-block pairwise ----------------
            r_all = sb.tile([128, nblk * FDB], F32, tag="r_all")
            for blk in range(nblk):
                (gx1_ps, gx2_ps, gy1_ps, gy2_ps), ag_ps = blk_tiles[blk]
                # prefetch next block's broadcasts onto the PE queue now
                if blk + 1 < nblk:
                    blk_tiles[blk + 1] = (bcast_coords(blk + 1), bcast_area(blk + 1))

                m2x = wrk.tile([128, FDB], F32, tag="m2x")
                nc.vector.tensor_scalar(m2x[:], gx1_ps[:], px1, None, Alu.max)
                zx = wrk.tile([128, FDB], F32, tag="zx")
                nc.vector.scalar_tensor_tensor(
                    zx[:], gx2_ps[:], px2, m2x[:], Alu.min, Alu.subtract
                )
                m2y = wrk.tile([128, FDB], F32, tag="m2y")
                nc.vector.tensor_scalar(m2y[:], gy1_ps[:], py1, None, Alu.max)
                zy = wrk.tile([128, FDB], F32, tag="zy")
                nc.vector.scalar_tensor_tensor(
                    zy[:], gy2_ps[:], py2, m2y[:], Alu.min, Alu.subtract
                )
                ihr = wrk.tile([128, FDB], F32, tag="ihr")
                nc.scalar.activation(ihr[:], zy[:], Act.Relu)
                S_sb = wrk.tile([128, FDB], F32, tag="S")
                nc.scalar.activation(
                    S_sb[:], ag_ps[:], Act.Identity, bias=ap_eps[:], scale=1.0
                )
                srecip = wrk.tile([128, FDB], F32, tag="srecip")
                nc.vector.reciprocal_approx_fast(srecip[:], S_sb[:])
                inter = wrk.tile([128, FDB], F32, tag="inter")
                nc.vector.scalar_tensor_tensor(
                    inter[:], zx[:], 0.0, ihr[:], Alu.max, Alu.mult
                )
                nc.vector.tensor_mul(
                    r_all[:, blk * FDB : (blk + 1) * FDB], inter[:], srecip[:]
                )

            fin = small.tile([128, 2], F32, tag="fin")
            nc.vector.tensor_reduce(fin[:, 0:1], um[:], mybir.AxisListType.X, Alu.max)
            rmax_prev = small.tile([128, 1], F32, tag="rmaxall")
            nc.vector.tensor_reduce(
                rmax_prev[:], r_all[:], mybir.AxisListType.X, Alu.max
            )

            # ---------------- final: iou = r/(1-r) per partition, one
            # fused cross-partition max over [umax | iou] ----------------
            one_m = small.tile([128, 1], F32, tag="one_m")
            nc.vector.tensor_scalar(
                one_m[:], rmax_prev[:], 1.0, -1.0, Alu.subtract, Alu.mult
            )
            rec = small.tile([128, 1], F32, tag="rec")
            nc.vector.reciprocal(rec[:], one_m[:])
            nc.vector.tensor_mul(fin[:, 1:2], rmax_prev[:], rec[:])
            fin_g = small.tile([128, 2], F32, tag="fin_g")
            nc.gpsimd.partition_all_reduce(
                fin_g[:], fin[:], 128, bass_isa.ReduceOp.max
            )
            nc.sync.dma_start(out=out.ap(), in_=fin_g[0:1, :])

    nc.compile()
    return nc


_KERNEL_CACHE = {}

# test/dev hooks
TRACE = False
LAST_RESULTS = None


def _get_kernel(K: int):
    if K not in _KERNEL_CACHE:
        _KERNEL_CACHE[K] = build_kernel(K)
    return _KERNEL_CACHE[K]


def make_in_maps(pred_boxes, pred_classes, gt_boxes, union_scores, union_classes):
    misc_shared = np.stack(
        [
            np.zeros(U, np.uint32),  # per-image, filled below
            union_scores.view(np.uint32),
            union_classes.view(np.uint32),
        ]
    )
    in_maps = []
    for b in range(B):
        ghi, glo = split_hi_lo(gt_boxes[b])
        phi, plo = split_hi_lo(pred_boxes[b])
        misc = misc_shared.copy()
        misc[0] = pred_classes[b].view(np.uint32)
        in_maps.append(
            {
                "misc": misc,
                "pb_hl": np.concatenate([phi, plo], axis=1),
                "gt_boxes": gt_boxes[b],
                "gt_hl": np.concatenate([ghi, glo], axis=1),
            }
        )
    return in_maps


def kernel(pred_boxes, pred_scores, pred_classes, gt_boxes, union_scores, union_classes):
    from concourse.bass_utils import run_bass_kernel_spmd

    pred_boxes = np.ascontiguousarray(np.asarray(pred_boxes, dtype=np.float32))
    pred_classes = np.ascontiguousarray(np.asarray(pred_classes, dtype=np.int32))
    gt_boxes = np.ascontiguousarray(np.asarray(gt_boxes, dtype=np.float32))
    union_scores = np.ascontiguousarray(np.asarray(union_scores, dtype=np.float32))
    union_classes = np.ascontiguousarray(np.asarray(union_classes, dtype=np.int32))

    max_persons = int((pred_classes == 0).sum(axis=1).max())
    K = 64 if max_persons <= 64 else 128
    nc = _get_kernel(K)

    in_maps = make_in_maps(pred_boxes, pred_classes, gt_boxes, union_scores, union_classes)
    res = run_bass_kernel_spmd(nc, in_maps, list(range(B)), trace=TRACE)
    global LAST_RESULTS
    LAST_RESULTS = res
    outs = np.stack([res.results[b]["out"] for b in range(B)])  # [B, 2]
    max_prob = outs[0, 0]
    max_iou = outs[:, 1].mean(dtype=np.float32)
    return np.array([max_prob, max_iou], dtype=np.float32)



# revision 8
# speedup vs baseline: 1.0936x; 1.0936x over previous
"""Trainium2 Bass kernel for nn_MaxExtractor (masked pairwise-IoU max + union max).

Contract: kernel(**inputs) takes FULL unsharded inputs, returns the FULL [2]
output. Internally shards the batch dim (8 images) across 8 NeuronCores, one
image per core; each core computes [max_prob_t, iou_max_of_its_image]; the
host gathers and averages the per-image iou scalars.

v2 design (per core, N=4096 preds, M=2048 gts, K=64 person slots):
  - All coordinates are shipped as fp16, centered by -IMG/2 so the fp16
    quantization step is <=0.25 over the whole canvas.
  - gt coord column-planes live in DRAM as [nhalf, 4*GTW]; ONE broadcast DMA
    (stride-0 partition dim) replicates each half across 64 partitions
    directly into SBUF [128, 4*GTW] fp16 - no PE broadcast, no PSUM.
  - Person compaction: class mask -> free-dim cumsum -> cross-partition
    prefix (triangular fp16 matmul) -> ranks -> 32x one-hot (fp16, 4x DVE
    mode) -> accumulating PE matmuls gather person boxes into [128, 4].
  - Pairwise runs all-SBUF fp16 (tensor_scalar at 4x, tensor_tensor at 2x):
    t1x = min(gx2,px2), m2x = max(gx1,px1), zx = t1x-m2x, same for y,
    inter = relu(zx)*relu(zy).
  - Ranking in log domain on the otherwise-idle Act engine:
    lnd = Ln(inter) - Ln(area_p + area_g), max-reduced; the single winner is
    mapped back via r = exp(lnd), iou = r/(1-r).  Areas for gt come from the
    broadcast planes (3 fp16 tensor_tensor ops).
  - Union max: fp16 mask-multiply + reduce.
  - Final: gpsimd partition all-reduce(max) over [umax | iou], 1 output DMA.
"""

import sys

sys.path.insert(0, "/opt/trn_rl_repo")

import contextlib

import numpy as np

import concourse.bacc as bacc
import concourse.mybir as mybir
from concourse import bass_isa
from concourse.tile import TileContext

F32 = mybir.dt.float32
F16 = mybir.dt.float16
U32 = mybir.dt.uint32
I32 = mybir.dt.int32
Alu = mybir.AluOpType
Act = mybir.ActivationFunctionType

N = 4096  # preds per image
M = 2048  # gts per image
B = 8  # images == cores
U = 4096  # union entries
CEN = 320.0  # coordinate centering offset (IMG/2)
EPS = 1.0e-9
NCH = 32  # pred chunks of 128 (compaction contract dim)
NPOOL_OH = 10  # one-hot chunks built on Pool (rest on DVE)


def build_kernel(K: int):
    """Build the per-core Bass module. K = person-slot count (64 or 128)."""
    assert K in (64, 128)
    nhalf = 128 // K  # gt halves packed along partitions
    GTW = M // nhalf  # gt columns per partition (1024 for K=64)
    HW = GTW // 2  # half-width for the pipelined ln/reduce stage

    nc = bacc.Bacc("TRN2", target_bir_lowering=False, debug=False)

    # host-packed inputs
    #   big u32 [128, 256]: cls(32) | pb_f16(64) | uscore_f16(16) | ucls_f16(16)
    #                      | tri_f16(64) | iota_f16(64)
    #   gt_cols f16 [nhalf, 4*GTW]: x1-plane | x2-plane | y1-plane | y2-plane
    big = nc.dram_tensor("big", [128, 256], U32, kind="ExternalInput")
    gt_cols = nc.dram_tensor("gt_cols", [nhalf, 4 * GTW], F16, kind="ExternalInput")
    out = nc.dram_tensor("out", [2], F32, kind="ExternalOutput")

    with TileContext(nc) as tc:
        ctx = contextlib.ExitStack()
        with ctx:
            sb = ctx.enter_context(tc.tile_pool(name="sbuf", bufs=1))
            wrk = ctx.enter_context(tc.tile_pool(name="wrk", bufs=2))
            ohp = ctx.enter_context(tc.tile_pool(name="ohp", bufs=32))
            small = ctx.enter_context(tc.tile_pool(name="small", bufs=1))
            ps_s = ctx.enter_context(tc.tile_pool(name="ps_s", bufs=2, space="PSUM"))

            # ---------------- loads ----------------
            bigt = sb.tile([128, 256], U32, tag="bigt")
            nc.sync.dma_start(out=bigt[:], in_=big.ap())
            cls_sb = bigt[:, 0:32].bitcast(I32)
            pb = bigt[:, 32:96].bitcast(F16)  # [128, 128] = 32 chunks x 4
            uscore = bigt[:, 96:112].bitcast(F16)  # [128, 32]
            ucls = bigt[:, 112:128].bitcast(F16)  # [128, 32]
            tri_sb = bigt[:, 128:192].bitcast(F16)  # [128, 128]
            iota_sb = bigt[:, 192:256].bitcast(F16)  # [128, 128], (j % K) + 1

            # gt broadcast: partition p reads half p // K of gt_cols
            gtb = sb.tile([128, 4 * GTW], F16, tag="gtb")
            for piece in range(2):  # split into 2 DMAs so x-planes land first
                c0, c1 = piece * 2 * GTW, (piece + 1) * 2 * GTW
                src = gt_cols.ap()[:, c0:c1].unsqueeze(1)
                src = src.broadcast_to([nhalf, K, 2 * GTW])
                (nc.scalar if piece == 0 else nc.sync).dma_start(
                    out=gtb[:, c0:c1], in_=src
                )
            gx1 = gtb[:, 0 * GTW : 1 * GTW]
            gx2 = gtb[:, 1 * GTW : 2 * GTW]
            gy1 = gtb[:, 2 * GTW : 3 * GTW]
            gy2 = gtb[:, 3 * GTW : 4 * GTW]

            # ---------------- person mask + ranks (DVE) ----------------
            m = small.tile([128, 32], F16, tag="m")
            nc.vector.tensor_scalar(m[:], cls_sb[:], 0, None, Alu.is_equal)
            s = small.tile([128, 32], F16, tag="s")
            nc.vector.tensor_tensor_scan(s[:], m[:], m[:], 0.0, Alu.add, Alu.max)
            pref_ps = ps_s.tile([128, 4], F32, tag="pss")
            nc.tensor.matmul(
                pref_ps[:, 0:1], tri_sb, s[:, 31:32], start=True, stop=True
            )
            q = small.tile([128, 32], F32, tag="q")
            nc.vector.scalar_tensor_tensor(
                q[:], s[:], pref_ps[:, 0:1], m[:], Alu.add, Alu.mult
            )

            # ---------------- compaction: one-hot + matmul gather -----------
            # oh[p, j] = (q[p, f] == iota[j]), iota[j] = (j % K) + 1
            pc_ps = ps_s.tile([128, 4], F32, tag="pss")
            for f in range(NCH):
                oh = ohp.tile([128, 128], F16, tag="oh")
                eng = nc.gpsimd if f < NPOOL_OH else nc.vector
                eng.tensor_scalar(
                    oh[:], iota_sb, q[:, f : f + 1], None, Alu.is_equal
                )
                nc.tensor.matmul(
                    pc_ps[:], oh[:], pb[:, 4 * f : 4 * f + 4],
                    start=(f == 0), stop=(f == NCH - 1),
                )
            pc = small.tile([128, 4], F32, tag="pcs")
            nc.vector.tensor_copy(pc[:], pc_ps[:])
            px1, py1, px2, py2 = (pc[:, i : i + 1] for i in range(4))
            wp = small.tile([128, 1], F32, tag="wp")
            nc.vector.tensor_sub(wp[:], px2, px1)
            hp = small.tile([128, 1], F32, tag="hp")
            nc.vector.tensor_sub(hp[:], py2, py1)
            ap_eps = small.tile([128, 1], F32, tag="ap_eps")
            nc.vector.scalar_tensor_tensor(
                ap_eps[:], wp[:], EPS, hp[:], Alu.bypass, Alu.mult
            )
            nc.vector.tensor_scalar(ap_eps[:], ap_eps[:], EPS, None, Alu.add)

            # ---------------- union max (fp16 mask-multiply) ----------------
            msk = small.tile([128, 32], F16, tag="msk")
            nc.vector.tensor_scalar(msk[:], ucls[:], 0.0, None, Alu.is_equal)
            sm = small.tile([128, 32], F16, tag="sm")
            nc.vector.tensor_mul(sm[:], msk[:], uscore[:])
            umax = small.tile([128, 1], F32, tag="umax")
            nc.vector.tensor_reduce(umax[:], sm[:], mybir.AxisListType.X, Alu.max)

            # ---------------- gt areas from broadcast planes ----------------
            wgb = wrk.tile([128, GTW], F16, tag="wgb")
            nc.vector.tensor_sub(wgb[:], gx2, gx1)
            hgb = wrk.tile([128, GTW], F16, tag="hgb")
            nc.vector.tensor_sub(hgb[:], gy2, gy1)
            agb = wrk.tile([128, GTW], F16, tag="agb")
            nc.vector.tensor_mul(agb[:], wgb[:], hgb[:])
            # lnS = Ln(agb + area_p) on Act, split in halves for pipelining
            lnS = wrk.tile([128, GTW], F16, tag="lnS")
            for h in range(2):
                nc.scalar.activation(
                    lnS[:, h * HW : (h + 1) * HW], agb[:, h * HW : (h + 1) * HW],
                    Act.Ln, bias=ap_eps[:], scale=1.0,
                )

            # ---------------- pairwise intersection (fp16, all SBUF) --------
            t1x = wrk.tile([128, GTW], F16, tag="t1x")
            nc.vector.tensor_scalar(t1x[:], gx2, px2, None, Alu.min)
            m2x = wrk.tile([128, GTW], F16, tag="m2x")
            nc.vector.tensor_scalar(m2x[:], gx1, px1, None, Alu.max)
            zx = wrk.tile([128, GTW], F16, tag="zx")
            nc.vector.tensor_sub(zx[:], t1x[:], m2x[:])
            t1y = wrk.tile([128, GTW], F16, tag="t1y")
            nc.vector.tensor_scalar(t1y[:], gy2, py2, None, Alu.min)
            m2y = wrk.tile([128, GTW], F16, tag="m2y")
            nc.vector.tensor_scalar(m2y[:], gy1, py1, None, Alu.max)
            zy = wrk.tile([128, GTW], F16, tag="zy")
            nc.vector.tensor_sub(zy[:], t1y[:], m2y[:])
            rx = wrk.tile([128, GTW], F16, tag="rx")
            nc.vector.tensor_scalar(rx[:], zx[:], 0.0, None, Alu.max)
            ry = wrk.tile([128, GTW], F16, tag="ry")
            nc.vector.tensor_scalar(ry[:], zy[:], 0.0, None, Alu.max)
            inter = wrk.tile([128, GTW], F16, tag="inter")
            nc.vector.tensor_mul(inter[:], rx[:], ry[:])

            # ---------------- ln(inter) - lnS, pipelined halves -------------
            lnI = wrk.tile([128, GTW], F16, tag="lnI")
            lnd = wrk.tile([128, GTW], F16, tag="lnd")
            red = small.tile([128, 2], F32, tag="red")
            for h in range(2):
                sl = slice(h * HW, (h + 1) * HW)
                nc.scalar.activation(lnI[:, sl], inter[:, sl], Act.Ln)
                nc.vector.tensor_sub(lnd[:, sl], lnI[:, sl], lnS[:, sl])
                nc.vector.tensor_reduce(
                    red[:, h : h + 1], lnd[:, sl], mybir.AxisListType.X, Alu.max
                )

            # ---------------- final: iou = r/(1-r), all-reduce --------------
            lmax = small.tile([128, 1], F32, tag="lmax")
            nc.vector.tensor_reduce(lmax[:], red[:], mybir.AxisListType.X, Alu.max)
            r = small.tile([128, 1], F32, tag="r")
            nc.scalar.activation(r[:], lmax[:], Act.Exp)
            one_m = small.tile([128, 1], F32, tag="one_m")
            nc.vector.tensor_scalar(
                one_m[:], r[:], 1.0, -1.0, Alu.subtract, Alu.mult
            )
            rec = small.tile([128, 1], F32, tag="rec")
            nc.vector.reciprocal(rec[:], one_m[:])
            fin = small.tile([128, 2], F32, tag="fin")
            nc.vector.tensor_copy(fin[:, 0:1], umax[:])
            nc.vector.tensor_mul(fin[:, 1:2], r[:], rec[:])
            fin_g = small.tile([128, 2], F32, tag="fin_g")
            nc.gpsimd.partition_all_reduce(
                fin_g[:], fin[:], 128, bass_isa.ReduceOp.max
            )
            nc.sync.dma_start(out=out.ap(), in_=fin_g[0:1, :])

    nc.compile()
    return nc


_KERNEL_CACHE = {}

# test/dev hooks
TRACE = False
LAST_RESULTS = None


def _get_kernel(K: int):
    if K not in _KERNEL_CACHE:
        _KERNEL_CACHE[K] = build_kernel(K)
    return _KERNEL_CACHE[K]


def make_in_maps(pred_boxes, pred_classes, gt_boxes, union_scores, union_classes, K):
    nhalf = 128 // K
    GTW = M // nhalf
    f16 = np.float16
    uscore16 = union_scores.astype(f16).reshape(128, 32)
    ucls16 = union_classes.astype(f16).reshape(128, 32)
    # iota: [128, 128], values (j % K) + 1 on every row
    iota = np.broadcast_to(
        np.tile(np.arange(1, K + 1, dtype=f16), 128 // K), (128, 128)
    )
    tri = (np.arange(128)[:, None] < np.arange(128)[None, :]).astype(f16)

    def pack_u32(a16):
        a16 = np.ascontiguousarray(a16)
        u16 = a16.view(np.uint16).astype(np.uint32)
        return u16[:, 0::2] | (u16[:, 1::2] << 16)

    iota_u32 = pack_u32(iota)  # [128, iw//2]
    tri_u32 = pack_u32(tri)  # [128, 64]
    us_u32 = pack_u32(uscore16)
    uc_u32 = pack_u32(ucls16)

    in_maps = []
    for b in range(B):
        pbc = (pred_boxes[b] - CEN).astype(f16).reshape(128, 128)  # 32 chunks x 4
        cls_u32 = pred_classes[b].reshape(128, 32).view(np.uint32)
        cols = [cls_u32, pack_u32(pbc), us_u32, uc_u32, tri_u32, iota_u32]
        bigarr = np.concatenate(cols, axis=1)
        assert bigarr.shape[1] == 256, bigarr.shape
        gtc = (gt_boxes[b] - CEN).astype(f16)  # [M, 4]
        # column planes per half: [nhalf, 4*GTW] = x1|x2|y1|y2
        gtc = gtc.reshape(nhalf, GTW, 4)
        gt_planes = np.concatenate(
            [gtc[:, :, 0], gtc[:, :, 2], gtc[:, :, 1], gtc[:, :, 3]], axis=1
        )
        in_maps.append(
            {
                "big": np.ascontiguousarray(bigarr.astype(np.uint32)),
                "gt_cols": np.ascontiguousarray(gt_planes),
            }
        )
    return in_maps


def kernel(pred_boxes, pred_scores, pred_classes, gt_boxes, union_scores, union_classes):
    from concourse.bass_utils import run_bass_kernel_spmd

    pred_boxes = np.ascontiguousarray(np.asarray(pred_boxes, dtype=np.float32))
    pred_classes = np.ascontiguousarray(np.asarray(pred_classes, dtype=np.int32))
    gt_boxes = np.ascontiguousarray(np.asarray(gt_boxes, dtype=np.float32))
    union_scores = np.ascontiguousarray(np.asarray(union_scores, dtype=np.float32))
    union_classes = np.ascontiguousarray(np.asarray(union_classes, dtype=np.int32))

    max_persons = int((pred_classes == 0).sum(axis=1).max())
    K = 64 if max_persons <= 64 else 128
    nc = _get_kernel(K)

    in_maps = make_in_maps(
        pred_boxes, pred_classes, gt_boxes, union_scores, union_classes, K
    )
    res = run_bass_kernel_spmd(nc, in_maps, list(range(B)), trace=TRACE)
    global LAST_RESULTS
    LAST_RESULTS = res
    outs = np.stack([res.results[b]["out"] for b in range(B)])  # [B, 2]
    max_prob = outs[0, 0]
    max_iou = outs[:, 1].mean(dtype=np.float32)
    return np.array([max_prob, max_iou], dtype=np.float32)


# revision 14
# speedup vs baseline: 1.2784x; 1.1689x over previous
"""Trainium2 Bass kernel for nn_MaxExtractor (masked pairwise-IoU max + union max).

Contract: kernel(**inputs) takes FULL unsharded inputs, returns the FULL [2]
output. Internally shards the batch dim (8 images) across 8 NeuronCores, one
image per core; each core computes [max_prob_t, iou_max_of_its_image]; the
host gathers and averages the per-image iou scalars.

v2 design (per core, N=4096 preds, M=2048 gts, K=64 person slots):
  - All coordinates are shipped as fp16, centered by -IMG/2 so the fp16
    quantization step is <=0.25 over the whole canvas.
  - gt coord column-planes live in DRAM as [nhalf, 4*GTW]; ONE broadcast DMA
    (stride-0 partition dim) replicates each half across 64 partitions
    directly into SBUF [128, 4*GTW] fp16 - no PE broadcast, no PSUM.
  - Person compaction: class mask -> free-dim cumsum -> cross-partition
    prefix (triangular fp16 matmul) -> ranks -> 32x one-hot (fp16, 4x DVE
    mode) -> accumulating PE matmuls gather person boxes into [128, 4].
  - Pairwise runs all-SBUF fp16 (tensor_scalar at 4x, tensor_tensor at 2x):
    t1x = min(gx2,px2), m2x = max(gx1,px1), zx = t1x-m2x, same for y,
    inter = relu(zx)*relu(zy).
  - Ranking in log domain on the otherwise-idle Act engine:
    lnd = Ln(inter) - Ln(area_p + area_g), max-reduced; the single winner is
    mapped back via r = exp(lnd), iou = r/(1-r).  Areas for gt come from the
    broadcast planes (3 fp16 tensor_tensor ops).
  - Union max: fp16 mask-multiply + reduce.
  - Final: gpsimd partition all-reduce(max) over [umax | iou], 1 output DMA.
"""

import sys

sys.path.insert(0, "/opt/trn_rl_repo")

import contextlib

import numpy as np

import concourse.bacc as bacc
import concourse.mybir as mybir
from concourse import bass_isa
from concourse.tile import TileContext

F32 = mybir.dt.float32
F16 = mybir.dt.float16
U32 = mybir.dt.uint32
I32 = mybir.dt.int32
Alu = mybir.AluOpType
Act = mybir.ActivationFunctionType

N = 4096  # preds per image
M = 2048  # gts per image
B = 8  # images == cores
U = 4096  # union entries
CEN = 320.0  # coordinate centering offset (IMG/2)
EPS = 1.0e-9
NCH = 32  # pred chunks of 128 (compaction contract dim)
NPOOL_OH = 10  # one-hot chunks built on Pool (rest on DVE)


def build_kernel(K: int):
    """Build the per-core Bass module. K = person-slot count (64 or 128)."""
    assert K in (64, 128)
    nhalf = 128 // K  # gt halves packed along partitions
    GTW = M // nhalf  # gt columns per partition (1024 for K=64)
    HW = GTW // 2  # half-width for the pipelined ln/reduce stage

    nc = bacc.Bacc("TRN2", target_bir_lowering=False, debug=False)

    # host-packed inputs
    #   big u32 [128, 256]: cls(32) | pb_f16(64) | uscore_f16(16) | ucls_f16(16)
    #                      | tri_f16(64) | iota_f16(64)
    #   gt_cols f16 [nhalf, 5*GTW]: x1 | x2 | y1 | y2 | area planes
    big = nc.dram_tensor("big", [128, 256], U32, kind="ExternalInput")
    gt_cols = nc.dram_tensor("gt_cols", [nhalf, 5 * GTW], F16, kind="ExternalInput")
    out = nc.dram_tensor("out", [128, 2], F32, kind="ExternalOutput")

    with TileContext(nc) as tc:
        ctx = contextlib.ExitStack()
        with ctx:
            sb = ctx.enter_context(tc.tile_pool(name="sbuf", bufs=1))
            wrk = ctx.enter_context(tc.tile_pool(name="wrk", bufs=2))
            ohp = ctx.enter_context(tc.tile_pool(name="ohp", bufs=32))
            small = ctx.enter_context(tc.tile_pool(name="small", bufs=1))
            ps_s = ctx.enter_context(tc.tile_pool(name="ps_s", bufs=2, space="PSUM"))

            # ---------------- loads ----------------
            bigt = sb.tile([128, 256], U32, tag="bigt")
            nc.sync.dma_start(out=bigt[:], in_=big.ap())
            cls_sb = bigt[:, 0:32].bitcast(I32)
            pb = bigt[:, 32:96].bitcast(F16)  # [128, 128] = 32 chunks x 4
            uscore = bigt[:, 96:112].bitcast(F16)  # [128, 32]
            ucls = bigt[:, 112:128].bitcast(F16)  # [128, 32]
            tri_sb = bigt[:, 128:192].bitcast(F16)  # [128, 128]
            iota_sb = bigt[:, 192:256].bitcast(F16)  # [128, 128], (j % K) + 1

            # gt broadcast: partition p reads half p // K of gt_cols
            gtb = sb.tile([128, 5 * GTW], F16, tag="gtb")
            src = gt_cols.ap().unsqueeze(1).broadcast_to([nhalf, K, 5 * GTW])
            nc.scalar.dma_start(out=gtb[:], in_=src)
            gx1 = gtb[:, 0 * GTW : 1 * GTW]
            gx2 = gtb[:, 1 * GTW : 2 * GTW]
            gy1 = gtb[:, 2 * GTW : 3 * GTW]
            gy2 = gtb[:, 3 * GTW : 4 * GTW]
            agb = gtb[:, 4 * GTW : 5 * GTW]

            # ---------------- person mask + ranks (DVE) ----------------
            m = small.tile([128, 32], F16, tag="m")
            nc.vector.tensor_scalar(m[:], cls_sb[:], 0, None, Alu.is_equal)
            s = small.tile([128, 32], F16, tag="s")
            nc.vector.tensor_tensor_scan(s[:], m[:], m[:], 0.0, Alu.add, Alu.max)
            pref_ps = ps_s.tile([128, 4], F32, tag="pss")
            nc.tensor.matmul(
                pref_ps[:, 0:1], tri_sb, s[:, 31:32], start=True, stop=True
            )
            q = small.tile([128, 32], F32, tag="q")
            nc.vector.scalar_tensor_tensor(
                q[:], s[:], pref_ps[:, 0:1], m[:], Alu.add, Alu.mult
            )

            # ---------------- compaction: one-hot + matmul gather -----------
            # oh[p, j] = (q[p, f] == iota[j]), iota[j] = (j % K) + 1
            # Last NPOOL_OH chunks build on Pool so PE's in-order accumulation
            # is never stalled by the slower Pool ops.
            pc_ps = ps_s.tile([128, 4], F32, tag="pss")
            for f in range(NCH):
                oh = ohp.tile([128, 128], F16, tag="oh")
                eng = nc.vector if f < NCH - NPOOL_OH else nc.gpsimd
                eng.tensor_scalar(
                    oh[:], iota_sb, q[:, f : f + 1], None, Alu.is_equal
                )
                nc.tensor.matmul(
                    pc_ps[:], oh[:], pb[:, 4 * f : 4 * f + 4],
                    start=(f == 0), stop=(f == NCH - 1),
                )
            pc = small.tile([128, 4], F32, tag="pcs")
            nc.vector.tensor_copy(pc[:], pc_ps[:])
            px1, py1, px2, py2 = (pc[:, i : i + 1] for i in range(4))
            wp = small.tile([128, 1], F32, tag="wp")
            nc.vector.tensor_sub(wp[:], px2, px1)
            hp = small.tile([128, 1], F32, tag="hp")
            nc.vector.tensor_sub(hp[:], py2, py1)
            ap_eps = small.tile([128, 1], F32, tag="ap_eps")
            nc.vector.scalar_tensor_tensor(
                ap_eps[:], wp[:], EPS, hp[:], Alu.bypass, Alu.mult
            )
            nc.vector.tensor_scalar(ap_eps[:], ap_eps[:], EPS, None, Alu.add)

            # ---------------- union max on Pool (fp16 mask-multiply) --------
            msk = small.tile([128, 32], F16, tag="msk")
            nc.gpsimd.tensor_scalar(msk[:], ucls[:], 0.0, None, Alu.is_equal)
            sm = small.tile([128, 32], F16, tag="sm")
            nc.gpsimd.tensor_mul(sm[:], msk[:], uscore[:])
            umax = small.tile([128, 1], F32, tag="umax")
            nc.vector.tensor_reduce(umax[:], sm[:], mybir.AxisListType.X, Alu.max)

            # lnS = Ln(agb + area_p) on Act, in quarters for pipelining
            QW = GTW // 4
            lnS = wrk.tile([128, GTW], F16, tag="lnS")
            for hq in range(4):
                sl = slice(hq * QW, (hq + 1) * QW)
                nc.scalar.activation(
                    lnS[:, sl], agb[:, sl], Act.Ln, bias=ap_eps[:], scale=1.0
                )

            # ---------------- pairwise intersection (fp16, all SBUF) --------
            t1x = wrk.tile([128, GTW], F16, tag="t1x")
            nc.vector.tensor_scalar(t1x[:], gx2, px2, None, Alu.min)
            m2x = wrk.tile([128, GTW], F16, tag="m2x")
            nc.vector.tensor_scalar(m2x[:], gx1, px1, None, Alu.max)
            zx = wrk.tile([128, GTW], F16, tag="zx")
            nc.vector.tensor_sub(zx[:], t1x[:], m2x[:])
            t1y = wrk.tile([128, GTW], F16, tag="t1y")
            nc.vector.tensor_scalar(t1y[:], gy2, py2, None, Alu.min)
            m2y = wrk.tile([128, GTW], F16, tag="m2y")
            nc.vector.tensor_scalar(m2y[:], gy1, py1, None, Alu.max)
            zy = wrk.tile([128, GTW], F16, tag="zy")
            nc.vector.tensor_sub(zy[:], t1y[:], m2y[:])
            rx = wrk.tile([128, GTW], F16, tag="rx")
            nc.vector.tensor_scalar(rx[:], zx[:], 0.0, None, Alu.max)
            ry = wrk.tile([128, GTW], F16, tag="ry")
            nc.vector.tensor_scalar(ry[:], zy[:], 0.0, None, Alu.max)
            inter = wrk.tile([128, GTW], F16, tag="inter")
            nc.vector.tensor_mul(inter[:], rx[:], ry[:])

            # ------------- ln(inter) - lnS, pipelined quarters --------------
            lnI = wrk.tile([128, GTW], F16, tag="lnI")
            lnd = wrk.tile([128, GTW], F16, tag="lnd")
            red = small.tile([128, 4], F32, tag="red")
            for hq in range(4):
                sl = slice(hq * QW, (hq + 1) * QW)
                nc.scalar.activation(lnI[:, sl], inter[:, sl], Act.Ln)
                nc.vector.tensor_sub(lnd[:, sl], lnI[:, sl], lnS[:, sl])
                nc.vector.tensor_reduce(
                    red[:, hq : hq + 1], lnd[:, sl], mybir.AxisListType.X, Alu.max
                )

            # --------- final: per-partition [umax | max lnd] to host --------
            fin = small.tile([128, 2], F32, tag="fin")
            nc.vector.tensor_copy(fin[:, 0:1], umax[:])
            nc.vector.tensor_reduce(
                fin[:, 1:2], red[:], mybir.AxisListType.X, Alu.max
            )
            nc.sync.dma_start(out=out.ap(), in_=fin[:])

    nc.compile()
    return nc


_KERNEL_CACHE = {}

# test/dev hooks
TRACE = False
LAST_RESULTS = None


def _get_kernel(K: int):
    if K not in _KERNEL_CACHE:
        _KERNEL_CACHE[K] = build_kernel(K)
    return _KERNEL_CACHE[K]


def make_in_maps(pred_boxes, pred_classes, gt_boxes, union_scores, union_classes, K):
    nhalf = 128 // K
    GTW = M // nhalf
    f16 = np.float16
    uscore16 = union_scores.astype(f16).reshape(128, 32)
    ucls16 = union_classes.astype(f16).reshape(128, 32)
    # iota: [128, 128], values (j % K) + 1 on every row
    iota = np.broadcast_to(
        np.tile(np.arange(1, K + 1, dtype=f16), 128 // K), (128, 128)
    )
    tri = (np.arange(128)[:, None] < np.arange(128)[None, :]).astype(f16)

    def pack_u32(a16):
        a16 = np.ascontiguousarray(a16)
        u16 = a16.view(np.uint16).astype(np.uint32)
        return u16[:, 0::2] | (u16[:, 1::2] << 16)

    iota_u32 = pack_u32(iota)  # [128, iw//2]
    tri_u32 = pack_u32(tri)  # [128, 64]
    us_u32 = pack_u32(uscore16)
    uc_u32 = pack_u32(ucls16)

    in_maps = []
    for b in range(B):
        pbc = (pred_boxes[b] - CEN).astype(f16).reshape(128, 128)  # 32 chunks x 4
        cls_u32 = pred_classes[b].reshape(128, 32).view(np.uint32)
        cols = [cls_u32, pack_u32(pbc), us_u32, uc_u32, tri_u32, iota_u32]
        bigarr = np.concatenate(cols, axis=1)
        assert bigarr.shape[1] == 256, bigarr.shape
        gtc = (gt_boxes[b] - CEN).astype(f16)  # [M, 4]
        ag = (
            (gt_boxes[b][:, 2] - gt_boxes[b][:, 0])
            * (gt_boxes[b][:, 3] - gt_boxes[b][:, 1])
        ).astype(f16)
        # column planes per half: [nhalf, 5*GTW] = x1|x2|y1|y2|area
        gtc = gtc.reshape(nhalf, GTW, 4)
        gt_planes = np.concatenate(
            [gtc[:, :, 0], gtc[:, :, 2], gtc[:, :, 1], gtc[:, :, 3],
             ag.reshape(nhalf, GTW)],
            axis=1,
        )
        in_maps.append(
            {
                "big": np.ascontiguousarray(bigarr.astype(np.uint32)),
                "gt_cols": np.ascontiguousarray(gt_planes),
            }
        )
    return in_maps


def kernel(pred_boxes, pred_scores, pred_classes, gt_boxes, union_scores, union_classes):
    from concourse.bass_utils import run_bass_kernel_spmd

    pred_boxes = np.ascontiguousarray(np.asarray(pred_boxes, dtype=np.float32))
    pred_classes = np.ascontiguousarray(np.asarray(pred_classes, dtype=np.int32))
    gt_boxes = np.ascontiguousarray(np.asarray(gt_boxes, dtype=np.float32))
    union_scores = np.ascontiguousarray(np.asarray(union_scores, dtype=np.float32))
    union_classes = np.ascontiguousarray(np.asarray(union_classes, dtype=np.int32))

    max_persons = int((pred_classes == 0).sum(axis=1).max())
    K = 64 if max_persons <= 64 else 128
    nc = _get_kernel(K)

    in_maps = make_in_maps(
        pred_boxes, pred_classes, gt_boxes, union_scores, union_classes, K
    )
    res = run_bass_kernel_spmd(nc, in_maps, list(range(B)), trace=TRACE)
    global LAST_RESULTS
    LAST_RESULTS = res
    outs = np.stack([res.results[b]["out"] for b in range(B)])  # [B, 128, 2]
    max_prob = outs[0, :, 0].max()
    lmax = outs[:, :, 1].max(axis=1)  # [B] per-image max of ln(inter/S)
    r = np.exp(lmax.astype(np.float64))
    iou = r / (1.0 - r)  # ln-rank back to iou = r/(1-r)
    max_iou = np.float32(iou.mean())
    return np.array([max_prob, max_iou], dtype=np.float32)


# revision 17
# speedup vs baseline: 1.2947x; 1.0128x over previous
"""Trainium2 Bass kernel for nn_MaxExtractor (masked pairwise-IoU max + union max).

Contract: kernel(**inputs) takes FULL unsharded inputs, returns the FULL [2]
output. Internally shards the batch dim (8 images) across 8 NeuronCores, one
image per core; each core computes [max_prob_t, iou_max_of_its_image]; the
host gathers and averages the per-image iou scalars.

v2 design (per core, N=4096 preds, M=2048 gts, K=64 person slots):
  - All coordinates are shipped as fp16, centered by -IMG/2 so the fp16
    quantization step is <=0.25 over the whole canvas.
  - gt coord column-planes live in DRAM as [nhalf, 4*GTW]; ONE broadcast DMA
    (stride-0 partition dim) replicates each half across 64 partitions
    directly into SBUF [128, 4*GTW] fp16 - no PE broadcast, no PSUM.
  - Person compaction: class mask -> free-dim cumsum -> cross-partition
    prefix (triangular fp16 matmul) -> ranks -> 32x one-hot (fp16, 4x DVE
    mode) -> accumulating PE matmuls gather person boxes into [128, 4].
  - Pairwise runs all-SBUF fp16 (tensor_scalar at 4x, tensor_tensor at 2x):
    t1x = min(gx2,px2), m2x = max(gx1,px1), zx = t1x-m2x, same for y,
    inter = relu(zx)*relu(zy).
  - Ranking in log domain on the otherwise-idle Act engine:
    lnd = Ln(inter) - Ln(area_p + area_g), max-reduced; the single winner is
    mapped back via r = exp(lnd), iou = r/(1-r).  Areas for gt come from the
    broadcast planes (3 fp16 tensor_tensor ops).
  - Union max: fp16 mask-multiply + reduce.
  - Final: gpsimd partition all-reduce(max) over [umax | iou], 1 output DMA.
"""

import sys

sys.path.insert(0, "/opt/trn_rl_repo")

import contextlib

import numpy as np

import concourse.bacc as bacc
import concourse.mybir as mybir
from concourse import bass_isa
from concourse.tile import TileContext

F32 = mybir.dt.float32
F16 = mybir.dt.float16
U32 = mybir.dt.uint32
I32 = mybir.dt.int32
Alu = mybir.AluOpType
Act = mybir.ActivationFunctionType

N = 4096  # preds per image
M = 2048  # gts per image
B = 8  # images == cores
U = 4096  # union entries
CEN = 320.0  # coordinate centering offset (IMG/2)
EPS = 1.0e-9
NCH = 32  # pred chunks of 128 (compaction contract dim)
NPOOL_OH = 8  # one-hot chunks built on Pool (rest on DVE)


def build_kernel(K: int):
    """Build the per-core Bass module. K = person-slot count (64 or 128)."""
    assert K in (64, 128)
    nhalf = 128 // K  # gt halves packed along partitions
    GTW = M // nhalf  # gt columns per partition (1024 for K=64)
    HW = GTW // 2  # half-width for the pipelined ln/reduce stage

    nc = bacc.Bacc("TRN2", target_bir_lowering=False, debug=False)

    # host-packed inputs
    #   big u32 [128, 256]: cls(32) | pb_f16(64) | uscore_f16(16) | ucls_f16(16)
    #                      | tri_f16(64) | iota_f16(64)
    #   gt_cols f16 [nhalf, 5*GTW]: x1 | x2 | y1 | y2 | area planes
    big = nc.dram_tensor("big", [128, 256], U32, kind="ExternalInput")
    gt_cols = nc.dram_tensor("gt_cols", [nhalf, 5 * GTW], F16, kind="ExternalInput")
    out = nc.dram_tensor("out", [128, 2], F32, kind="ExternalOutput")

    with TileContext(nc) as tc:
        ctx = contextlib.ExitStack()
        with ctx:
            sb = ctx.enter_context(tc.tile_pool(name="sbuf", bufs=1))
            wrk = ctx.enter_context(tc.tile_pool(name="wrk", bufs=2))
            ohp = ctx.enter_context(tc.tile_pool(name="ohp", bufs=32))
            small = ctx.enter_context(tc.tile_pool(name="small", bufs=1))
            ps_s = ctx.enter_context(tc.tile_pool(name="ps_s", bufs=2, space="PSUM"))

            # ---------------- loads ----------------
            bigt = sb.tile([128, 256], U32, tag="bigt")
            nc.sync.dma_start(out=bigt[:], in_=big.ap())
            cls_sb = bigt[:, 0:32].bitcast(I32)
            pb = bigt[:, 32:96].bitcast(F16)  # [128, 128] = 32 chunks x 4
            uscore = bigt[:, 96:112].bitcast(F16)  # [128, 32]
            ucls = bigt[:, 112:128].bitcast(F16)  # [128, 32]
            tri_sb = bigt[:, 128:192].bitcast(F16)  # [128, 128]
            iota_sb = bigt[:, 192:256].bitcast(F16)  # [128, 128], (j % K) + 1

            # gt broadcast: partition p reads half p // K of gt_cols
            gtb = sb.tile([128, 5 * GTW], F16, tag="gtb")
            src = gt_cols.ap().unsqueeze(1).broadcast_to([nhalf, K, 5 * GTW])
            nc.sync.dma_start(out=gtb[:], in_=src)

            # preload the Ln activation table while DMAs are in flight: a
            # dummy Ln on a memset tile adopts the table-load so the real
            # lnS/lnI calls don't pay the 1283ns load on the critical path
            dmy = small.tile([128, 1], F32, tag="dmy")
            nc.vector.memset(dmy[:], 1.0)
            dmy2 = small.tile([128, 1], F32, tag="dmy2")
            nc.scalar.activation(dmy2[:], dmy[:], Act.Ln)
            gx1 = gtb[:, 0 * GTW : 1 * GTW]
            gx2 = gtb[:, 1 * GTW : 2 * GTW]
            gy1 = gtb[:, 2 * GTW : 3 * GTW]
            gy2 = gtb[:, 3 * GTW : 4 * GTW]
            agb = gtb[:, 4 * GTW : 5 * GTW]

            # ---------------- person mask + ranks (DVE) ----------------
            m = small.tile([128, 32], F16, tag="m")
            nc.vector.tensor_scalar(m[:], cls_sb[:], 0, None, Alu.is_equal)
            s = small.tile([128, 32], F16, tag="s")
            nc.vector.tensor_tensor_scan(s[:], m[:], m[:], 0.0, Alu.add, Alu.max)
            pref_ps = ps_s.tile([128, 4], F32, tag="pss")
            nc.tensor.matmul(
                pref_ps[:, 0:1], tri_sb, s[:, 31:32], start=True, stop=True
            )
            q = small.tile([128, 32], F32, tag="q")
            nc.vector.scalar_tensor_tensor(
                q[:], s[:], pref_ps[:, 0:1], m[:], Alu.add, Alu.mult
            )

            # ---------------- compaction: one-hot + matmul gather -----------
            # oh[p, j] = (q[p, f] == iota[j]), iota[j] = (j % K) + 1
            # Last NPOOL_OH chunks build on Pool so PE's in-order accumulation
            # is never stalled by the slower Pool ops.
            pc_ps = ps_s.tile([128, 4], F32, tag="pss")
            for f in range(NCH):
                oh = ohp.tile([128, 128], F16, tag="oh")
                eng = nc.vector if f < NCH - NPOOL_OH else nc.gpsimd
                eng.tensor_scalar(
                    oh[:], iota_sb, q[:, f : f + 1], None, Alu.is_equal
                )
                nc.tensor.matmul(
                    pc_ps[:], oh[:], pb[:, 4 * f : 4 * f + 4],
                    start=(f == 0), stop=(f == NCH - 1),
                )
            pc = small.tile([128, 4], F32, tag="pcs")
            nc.vector.tensor_copy(pc[:], pc_ps[:])
            px1, py1, px2, py2 = (pc[:, i : i + 1] for i in range(4))
            wp = small.tile([128, 1], F32, tag="wp")
            nc.vector.tensor_sub(wp[:], px2, px1)
            hp = small.tile([128, 1], F32, tag="hp")
            nc.vector.tensor_sub(hp[:], py2, py1)
            ap_eps = small.tile([128, 1], F32, tag="ap_eps")
            nc.vector.scalar_tensor_tensor(
                ap_eps[:], wp[:], EPS, hp[:], Alu.bypass, Alu.mult
            )
            nc.vector.tensor_scalar(ap_eps[:], ap_eps[:], EPS, None, Alu.add)

            # ------- union max on Pool, after its one-hot chunks ------------
            msk = small.tile([128, 32], F16, tag="msk")
            nc.gpsimd.tensor_scalar(msk[:], ucls[:], 0.0, None, Alu.is_equal)
            sm = small.tile([128, 32], F16, tag="sm")
            nc.gpsimd.tensor_mul(sm[:], msk[:], uscore[:])
            umax = small.tile([128, 1], F32, tag="umax")
            nc.vector.tensor_reduce(umax[:], sm[:], mybir.AxisListType.X, Alu.max)

            # lnS = Ln(agb + area_p) on Act, in quarters for pipelining
            QW = GTW // 4
            lnS = wrk.tile([128, GTW], F16, tag="lnS")
            for hq in range(4):
                sl = slice(hq * QW, (hq + 1) * QW)
                nc.scalar.activation(
                    lnS[:, sl], agb[:, sl], Act.Ln, bias=ap_eps[:], scale=1.0
                )

            # ---------------- pairwise intersection (fp16, all SBUF) --------
            t1x = wrk.tile([128, GTW], F16, tag="t1x")
            nc.vector.tensor_scalar(t1x[:], gx2, px2, None, Alu.min)
            m2x = wrk.tile([128, GTW], F16, tag="m2x")
            nc.vector.tensor_scalar(m2x[:], gx1, px1, None, Alu.max)
            zx = wrk.tile([128, GTW], F16, tag="zx")
            nc.vector.tensor_sub(zx[:], t1x[:], m2x[:])
            t1y = wrk.tile([128, GTW], F16, tag="t1y")
            nc.vector.tensor_scalar(t1y[:], gy2, py2, None, Alu.min)
            m2y = wrk.tile([128, GTW], F16, tag="m2y")
            nc.vector.tensor_scalar(m2y[:], gy1, py1, None, Alu.max)
            zy = wrk.tile([128, GTW], F16, tag="zy")
            nc.vector.tensor_sub(zy[:], t1y[:], m2y[:])
            rx = wrk.tile([128, GTW], F16, tag="rx")
            nc.vector.tensor_scalar(rx[:], zx[:], 0.0, None, Alu.max)
            ry = wrk.tile([128, GTW], F16, tag="ry")
            nc.vector.tensor_scalar(ry[:], zy[:], 0.0, None, Alu.max)
            inter = wrk.tile([128, GTW], F16, tag="inter")
            nc.vector.tensor_mul(inter[:], rx[:], ry[:])

            # ------------- ln(inter) - lnS, pipelined quarters --------------
            lnI = wrk.tile([128, GTW], F16, tag="lnI")
            lnd = wrk.tile([128, GTW], F16, tag="lnd")
            red = small.tile([128, 4], F32, tag="red")
            for hq in range(4):
                sl = slice(hq * QW, (hq + 1) * QW)
                nc.scalar.activation(lnI[:, sl], inter[:, sl], Act.Ln)
                nc.vector.tensor_sub(lnd[:, sl], lnI[:, sl], lnS[:, sl])
                nc.vector.tensor_reduce(
                    red[:, hq : hq + 1], lnd[:, sl], mybir.AxisListType.X, Alu.max
                )

            # --------- final: per-partition [umax | max lnd] to host --------
            fin = small.tile([128, 2], F32, tag="fin")
            nc.vector.tensor_copy(fin[:, 0:1], umax[:])
            nc.vector.tensor_reduce(
                fin[:, 1:2], red[:], mybir.AxisListType.X, Alu.max
            )
            nc.sync.dma_start(out=out.ap(), in_=fin[:])

    nc.compile()
    return nc


_KERNEL_CACHE = {}

# test/dev hooks
TRACE = False
LAST_RESULTS = None


def _get_kernel(K: int):
    if K not in _KERNEL_CACHE:
        _KERNEL_CACHE[K] = build_kernel(K)
    return _KERNEL_CACHE[K]


def make_in_maps(pred_boxes, pred_classes, gt_boxes, union_scores, union_classes, K):
    nhalf = 128 // K
    GTW = M // nhalf
    f16 = np.float16
    uscore16 = union_scores.astype(f16).reshape(128, 32)
    ucls16 = union_classes.astype(f16).reshape(128, 32)
    # iota: [128, 128], values (j % K) + 1 on every row
    iota = np.broadcast_to(
        np.tile(np.arange(1, K + 1, dtype=f16), 128 // K), (128, 128)
    )
    tri = (np.arange(128)[:, None] < np.arange(128)[None, :]).astype(f16)

    def pack_u32(a16):
        a16 = np.ascontiguousarray(a16)
        u16 = a16.view(np.uint16).astype(np.uint32)
        return u16[:, 0::2] | (u16[:, 1::2] << 16)

    iota_u32 = pack_u32(iota)  # [128, iw//2]
    tri_u32 = pack_u32(tri)  # [128, 64]
    us_u32 = pack_u32(uscore16)
    uc_u32 = pack_u32(ucls16)

    in_maps = []
    for b in range(B):
        pbc = (pred_boxes[b] - CEN).astype(f16).reshape(128, 128)  # 32 chunks x 4
        cls_u32 = pred_classes[b].reshape(128, 32).view(np.uint32)
        cols = [cls_u32, pack_u32(pbc), us_u32, uc_u32, tri_u32, iota_u32]
        bigarr = np.concatenate(cols, axis=1)
        assert bigarr.shape[1] == 256, bigarr.shape
        gtc = (gt_boxes[b] - CEN).astype(f16)  # [M, 4]
        ag = (
            (gt_boxes[b][:, 2] - gt_boxes[b][:, 0])
            * (gt_boxes[b][:, 3] - gt_boxes[b][:, 1])
        ).astype(f16)
        # column planes per half: [nhalf, 5*GTW] = x1|x2|y1|y2|area
        gtc = gtc.reshape(nhalf, GTW, 4)
        gt_planes = np.concatenate(
            [gtc[:, :, 0], gtc[:, :, 2], gtc[:, :, 1], gtc[:, :, 3],
             ag.reshape(nhalf, GTW)],
            axis=1,
        )
        in_maps.append(
            {
                "big": np.ascontiguousarray(bigarr.astype(np.uint32)),
                "gt_cols": np.ascontiguousarray(gt_planes),
            }
        )
    return in_maps


def kernel(pred_boxes, pred_scores, pred_classes, gt_boxes, union_scores, union_classes):
    from concourse.bass_utils import run_bass_kernel_spmd

    pred_boxes = np.ascontiguousarray(np.asarray(pred_boxes, dtype=np.float32))
    pred_classes = np.ascontiguousarray(np.asarray(pred_classes, dtype=np.int32))
    gt_boxes = np.ascontiguousarray(np.asarray(gt_boxes, dtype=np.float32))
    union_scores = np.ascontiguousarray(np.asarray(union_scores, dtype=np.float32))
    union_classes = np.ascontiguousarray(np.asarray(union_classes, dtype=np.int32))

    max_persons = int((pred_classes == 0).sum(axis=1).max())
    K = 64 if max_persons <= 64 else 128
    nc = _get_kernel(K)

    in_maps = make_in_maps(
        pred_boxes, pred_classes, gt_boxes, union_scores, union_classes, K
    )
    res = run_bass_kernel_spmd(nc, in_maps, list(range(B)), trace=TRACE)
    global LAST_RESULTS
    LAST_RESULTS = res
    outs = np.stack([res.results[b]["out"] for b in range(B)])  # [B, 128, 2]
    max_prob = outs[0, :, 0].max()
    lmax = outs[:, :, 1].max(axis=1)  # [B] per-image max of ln(inter/S)
    r = np.exp(lmax.astype(np.float64))
    iou = r / (1.0 - r)  # ln-rank back to iou = r/(1-r)
    max_iou = np.float32(iou.mean())
    return np.array([max_prob, max_iou], dtype=np.float32)


# revision 21
# speedup vs baseline: 1.3467x; 1.0401x over previous
"""Trainium2 Bass kernel for nn_MaxExtractor (masked pairwise-IoU max + union max).

Contract: kernel(**inputs) takes FULL unsharded inputs, returns the FULL [2]
output. Internally shards the batch dim (8 images) across 8 NeuronCores, one
image per core; each core computes [max_prob_t, iou_max_of_its_image]; the
host gathers and averages the per-image iou scalars.

v2 design (per core, N=4096 preds, M=2048 gts, K=64 person slots):
  - All coordinates are shipped as fp16, centered by -IMG/2 so the fp16
    quantization step is <=0.25 over the whole canvas.
  - gt coord column-planes live in DRAM as [nhalf, 4*GTW]; ONE broadcast DMA
    (stride-0 partition dim) replicates each half across 64 partitions
    directly into SBUF [128, 4*GTW] fp16 - no PE broadcast, no PSUM.
  - Person compaction: class mask -> free-dim cumsum -> cross-partition
    prefix (triangular fp16 matmul) -> ranks -> 32x one-hot (fp16, 4x DVE
    mode) -> accumulating PE matmuls gather person boxes into [128, 4].
  - Pairwise runs all-SBUF fp16 (tensor_scalar at 4x, tensor_tensor at 2x):
    t1x = min(gx2,px2), m2x = max(gx1,px1), zx = t1x-m2x, same for y,
    inter = relu(zx)*relu(zy).
  - Ranking in log domain on the otherwise-idle Act engine:
    lnd = Ln(inter) - Ln(area_p + area_g), max-reduced; the single winner is
    mapped back via r = exp(lnd), iou = r/(1-r).  Areas for gt come from the
    broadcast planes (3 fp16 tensor_tensor ops).
  - Union max: fp16 mask-multiply + reduce.
  - Final: gpsimd partition all-reduce(max) over [umax | iou], 1 output DMA.
"""

import sys

sys.path.insert(0, "/opt/trn_rl_repo")

import contextlib

import numpy as np

import concourse.bacc as bacc
import concourse.mybir as mybir
from concourse import bass_isa
from concourse.tile import TileContext

F32 = mybir.dt.float32
F16 = mybir.dt.float16
U32 = mybir.dt.uint32
I32 = mybir.dt.int32
Alu = mybir.AluOpType
Act = mybir.ActivationFunctionType

N = 4096  # preds per image
M = 2048  # gts per image
B = 8  # images == cores
U = 4096  # union entries
CEN = 320.0  # coordinate centering offset (IMG/2)
EPS = 1.0e-9
NCH = 32  # pred chunks of 128 (compaction contract dim)
NPOOL_OH = 8  # one-hot chunks built on Pool (rest on DVE)


def build_kernel(K: int):
    """Build the per-core Bass module. K = person-slot count (64 or 128)."""
    assert K in (64, 128)
    nhalf = 128 // K  # gt halves packed along partitions
    GTW = M // nhalf  # gt columns per partition (1024 for K=64)
    HW = GTW // 2  # half-width for the pipelined ln/reduce stage

    nc = bacc.Bacc("TRN2", target_bir_lowering=False, debug=False)

    # host-packed inputs
    #   big u32 [128, 256]: cls(32) | pb_f16(64) | uscore_f16(16) | ucls_f16(16)
    #                      | tri_f16(64) | iota_f16(64)
    #   gt_cols f16 [nhalf, 5*GTW]: x1 | x2 | y1 | y2 | area planes
    big = nc.dram_tensor("big", [128, 256], U32, kind="ExternalInput")
    gt_cols = nc.dram_tensor("gt_cols", [nhalf, 5 * GTW], F16, kind="ExternalInput")
    out = nc.dram_tensor("out", [128, 2], F32, kind="ExternalOutput")

    with TileContext(nc) as tc:
        ctx = contextlib.ExitStack()
        with ctx:
            sb = ctx.enter_context(tc.tile_pool(name="sbuf", bufs=1))
            wrk = ctx.enter_context(tc.tile_pool(name="wrk", bufs=2))
            ohp = ctx.enter_context(tc.tile_pool(name="ohp", bufs=32))
            small = ctx.enter_context(tc.tile_pool(name="small", bufs=1))
            ps_s = ctx.enter_context(tc.tile_pool(name="ps_s", bufs=2, space="PSUM"))

            # ---------------- loads ----------------
            bigt = sb.tile([128, 256], U32, tag="bigt")
            nc.sync.dma_start(out=bigt[:], in_=big.ap())
            cls_sb = bigt[:, 0:32].bitcast(I32)
            pb = bigt[:, 32:96].bitcast(F16)  # [128, 128] = 32 chunks x 4
            uscore = bigt[:, 96:112].bitcast(F16)  # [128, 32]
            ucls = bigt[:, 112:128].bitcast(F16)  # [128, 32]
            tri_sb = bigt[:, 128:192].bitcast(F16)  # [128, 128]
            iota_sb = bigt[:, 192:256].bitcast(F16)  # [128, 128], (j % K) + 1

            # gt broadcast: partition p reads half p // K of gt_cols.
            # Coord planes and the area plane ship as separate DMAs so the
            # pairwise chain can start before the area transfer completes.
            gtb = sb.tile([128, 5 * GTW], F16, tag="gtb")
            src_xy = gt_cols.ap()[:, 0 : 4 * GTW].unsqueeze(1)
            nc.sync.dma_start(
                out=gtb[:, 0 : 4 * GTW],
                in_=src_xy.broadcast_to([nhalf, K, 4 * GTW]),
            )
            src_ag = gt_cols.ap()[:, 4 * GTW : 5 * GTW].unsqueeze(1)
            nc.sync.dma_start(
                out=gtb[:, 4 * GTW : 5 * GTW],
                in_=src_ag.broadcast_to([nhalf, K, GTW]),
            )

            # preload the Ln activation table while DMAs are in flight: a
            # dummy Ln on a memset tile adopts the table-load so the real
            # lnS/lnI calls don't pay the 1283ns load on the critical path
            dmy = small.tile([128, 1], F32, tag="dmy")
            nc.vector.memset(dmy[:], 1.0)
            dmy2 = small.tile([128, 1], F32, tag="dmy2")
            nc.scalar.activation(dmy2[:], dmy[:], Act.Ln)
            gx1 = gtb[:, 0 * GTW : 1 * GTW]
            gx2 = gtb[:, 1 * GTW : 2 * GTW]
            gy1 = gtb[:, 2 * GTW : 3 * GTW]
            gy2 = gtb[:, 3 * GTW : 4 * GTW]
            agb = gtb[:, 4 * GTW : 5 * GTW]

            # ---------------- person mask + ranks (DVE) ----------------
            m = small.tile([128, 32], F16, tag="m")
            nc.vector.tensor_scalar(m[:], cls_sb[:], 0, None, Alu.is_equal)
            s = small.tile([128, 32], F16, tag="s")
            nc.vector.tensor_tensor_scan(s[:], m[:], m[:], 0.0, Alu.add, Alu.max)
            pref_ps = ps_s.tile([128, 4], F32, tag="pss")
            nc.tensor.matmul(
                pref_ps[:, 0:1], tri_sb, s[:, 31:32], start=True, stop=True
            )
            q = small.tile([128, 32], F32, tag="q")
            nc.vector.scalar_tensor_tensor(
                q[:], s[:], pref_ps[:, 0:1], m[:], Alu.add, Alu.mult
            )

            # ---------------- compaction: one-hot + matmul gather -----------
            # oh[p, j] = (q[p, f] == iota[j]), iota[j] = (j % K) + 1
            # Last NPOOL_OH chunks build on Pool so PE's in-order accumulation
            # is never stalled by the slower Pool ops.
            pc_ps = ps_s.tile([128, 4], F32, tag="pss")
            for f in range(NCH):
                oh = ohp.tile([128, 128], F16, tag="oh")
                eng = nc.gpsimd if f < NPOOL_OH else nc.vector
                eng.tensor_scalar(
                    oh[:], iota_sb, q[:, f : f + 1], None, Alu.is_equal
                )
                nc.tensor.matmul(
                    pc_ps[:], oh[:], pb[:, 4 * f : 4 * f + 4],
                    start=(f == 0), stop=(f == NCH - 1),
                )
            pc = small.tile([128, 4], F32, tag="pcs")
            nc.vector.tensor_copy(pc[:], pc_ps[:])
            px1, py1, px2, py2 = (pc[:, i : i + 1] for i in range(4))
            wp = small.tile([128, 1], F32, tag="wp")
            nc.vector.tensor_sub(wp[:], px2, px1)
            hp = small.tile([128, 1], F32, tag="hp")
            nc.vector.tensor_sub(hp[:], py2, py1)
            ap_eps = small.tile([128, 1], F32, tag="ap_eps")
            nc.vector.scalar_tensor_tensor(
                ap_eps[:], wp[:], EPS, hp[:], Alu.bypass, Alu.mult
            )
            nc.vector.tensor_scalar(ap_eps[:], ap_eps[:], EPS, None, Alu.add)

            # lnS = Ln(agb + area_p) on Act, in quarters for pipelining
            QW = GTW // 4
            lnS = wrk.tile([128, GTW], F16, tag="lnS")
            for hq in range(4):
                sl = slice(hq * QW, (hq + 1) * QW)
                nc.scalar.activation(
                    lnS[:, sl], agb[:, sl], Act.Ln, bias=ap_eps[:], scale=1.0
                )

            # ---------------- pairwise intersection (fp16, all SBUF) --------
            t1x = wrk.tile([128, GTW], F16, tag="t1x")
            nc.vector.tensor_scalar(t1x[:], gx2, px2, None, Alu.min)
            m2x = wrk.tile([128, GTW], F16, tag="m2x")
            nc.vector.tensor_scalar(m2x[:], gx1, px1, None, Alu.max)
            zx = wrk.tile([128, GTW], F16, tag="zx")
            nc.vector.tensor_sub(zx[:], t1x[:], m2x[:])
            t1y = wrk.tile([128, GTW], F16, tag="t1y")
            nc.vector.tensor_scalar(t1y[:], gy2, py2, None, Alu.min)
            m2y = wrk.tile([128, GTW], F16, tag="m2y")
            nc.vector.tensor_scalar(m2y[:], gy1, py1, None, Alu.max)
            zy = wrk.tile([128, GTW], F16, tag="zy")
            nc.vector.tensor_sub(zy[:], t1y[:], m2y[:])
            # rx/ry/inter per half so the Act ln chain starts earlier;
            # lnI/lnd/reduce per quarter to pipeline Act against DVE
            rx = wrk.tile([128, GTW], F16, tag="rx")
            ry = wrk.tile([128, GTW], F16, tag="ry")
            inter = wrk.tile([128, GTW], F16, tag="inter")
            lnI = wrk.tile([128, GTW], F16, tag="lnI")
            lnd = wrk.tile([128, GTW], F16, tag="lnd")
            red = small.tile([128, 4], F32, tag="red")
            for h in range(2):
                hs = slice(h * HW, (h + 1) * HW)
                nc.vector.tensor_scalar(rx[:, hs], zx[:, hs], 0.0, None, Alu.max)
                nc.vector.tensor_scalar(ry[:, hs], zy[:, hs], 0.0, None, Alu.max)
                nc.vector.tensor_mul(inter[:, hs], rx[:, hs], ry[:, hs])
                for qq in range(2):
                    hq = 2 * h + qq
                    sl = slice(hq * QW, (hq + 1) * QW)
                    nc.scalar.activation(lnI[:, sl], inter[:, sl], Act.Ln)
                    nc.vector.tensor_sub(lnd[:, sl], lnI[:, sl], lnS[:, sl])
                    nc.vector.tensor_reduce(
                        red[:, hq : hq + 1], lnd[:, sl],
                        mybir.AxisListType.X, Alu.max,
                    )

            # ------- union max in the gtb DMA-wait bubble on DVE ------------
            msk = small.tile([128, 32], F16, tag="msk")
            nc.vector.tensor_scalar(msk[:], ucls[:], 0.0, None, Alu.is_equal)
            sm = small.tile([128, 32], F16, tag="sm")
            nc.vector.tensor_mul(sm[:], msk[:], uscore[:])
            umax = small.tile([128, 1], F32, tag="umax")
            nc.vector.tensor_reduce(umax[:], sm[:], mybir.AxisListType.X, Alu.max)

            # --------- final: per-partition [umax | max lnd] to host --------
            fin = small.tile([128, 2], F32, tag="fin")
            nc.vector.tensor_copy(fin[:, 0:1], umax[:])
            nc.vector.tensor_reduce(
                fin[:, 1:2], red[:], mybir.AxisListType.X, Alu.max
            )
            nc.sync.dma_start(out=out.ap(), in_=fin[:])

    nc.compile()
    return nc


_KERNEL_CACHE = {}

# test/dev hooks
TRACE = False
LAST_RESULTS = None


def _get_kernel(K: int):
    if K not in _KERNEL_CACHE:
        _KERNEL_CACHE[K] = build_kernel(K)
    return _KERNEL_CACHE[K]


def make_in_maps(pred_boxes, pred_classes, gt_boxes, union_scores, union_classes, K):
    nhalf = 128 // K
    GTW = M // nhalf
    f16 = np.float16
    uscore16 = union_scores.astype(f16).reshape(128, 32)
    ucls16 = union_classes.astype(f16).reshape(128, 32)
    # iota: [128, 128], values (j % K) + 1 on every row
    iota = np.broadcast_to(
        np.tile(np.arange(1, K + 1, dtype=f16), 128 // K), (128, 128)
    )
    tri = (np.arange(128)[:, None] < np.arange(128)[None, :]).astype(f16)

    def pack_u32(a16):
        a16 = np.ascontiguousarray(a16)
        u16 = a16.view(np.uint16).astype(np.uint32)
        return u16[:, 0::2] | (u16[:, 1::2] << 16)

    iota_u32 = pack_u32(iota)  # [128, iw//2]
    tri_u32 = pack_u32(tri)  # [128, 64]
    us_u32 = pack_u32(uscore16)
    uc_u32 = pack_u32(ucls16)

    in_maps = []
    for b in range(B):
        pbc = (pred_boxes[b] - CEN).astype(f16).reshape(128, 128)  # 32 chunks x 4
        cls_u32 = pred_classes[b].reshape(128, 32).view(np.uint32)
        cols = [cls_u32, pack_u32(pbc), us_u32, uc_u32, tri_u32, iota_u32]
        bigarr = np.concatenate(cols, axis=1)
        assert bigarr.shape[1] == 256, bigarr.shape
        gtc = (gt_boxes[b] - CEN).astype(f16)  # [M, 4]
        ag = (
            (gt_boxes[b][:, 2] - gt_boxes[b][:, 0])
            * (gt_boxes[b][:, 3] - gt_boxes[b][:, 1])
        ).astype(f16)
        # column planes per half: [nhalf, 5*GTW] = x1|x2|y1|y2|area
        gtc = gtc.reshape(nhalf, GTW, 4)
        gt_planes = np.concatenate(
            [gtc[:, :, 0], gtc[:, :, 2], gtc[:, :, 1], gtc[:, :, 3],
             ag.reshape(nhalf, GTW)],
            axis=1,
        )
        in_maps.append(
            {
                "big": np.ascontiguousarray(bigarr.astype(np.uint32)),
                "gt_cols": np.ascontiguousarray(gt_planes),
            }
        )
    return in_maps


def kernel(pred_boxes, pred_scores, pred_classes, gt_boxes, union_scores, union_classes):
    from concourse.bass_utils import run_bass_kernel_spmd

    pred_boxes = np.ascontiguousarray(np.asarray(pred_boxes, dtype=np.float32))
    pred_classes = np.ascontiguousarray(np.asarray(pred_classes, dtype=np.int32))
    gt_boxes = np.ascontiguousarray(np.asarray(gt_boxes, dtype=np.float32))
    union_scores = np.ascontiguousarray(np.asarray(union_scores, dtype=np.float32))
    union_classes = np.ascontiguousarray(np.asarray(union_classes, dtype=np.int32))

    max_persons = int((pred_classes == 0).sum(axis=1).max())
    K = 64 if max_persons <= 64 else 128
    nc = _get_kernel(K)

    in_maps = make_in_maps(
        pred_boxes, pred_classes, gt_boxes, union_scores, union_classes, K
    )
    res = run_bass_kernel_spmd(nc, in_maps, list(range(B)), trace=TRACE)
    global LAST_RESULTS
    LAST_RESULTS = res
    outs = np.stack([res.results[b]["out"] for b in range(B)])  # [B, 128, 2]
    max_prob = outs[0, :, 0].max()
    lmax = outs[:, :, 1].max(axis=1)  # [B] per-image max of ln(inter/S)
    r = np.exp(lmax.astype(np.float64))
    iou = r / (1.0 - r)  # ln-rank back to iou = r/(1-r)
    max_iou = np.float32(iou.mean())
    return np.array([max_prob, max_iou], dtype=np.float32)
